# revision 1
# baseline (speedup 1.0000x reference)
import sys

if "/opt/trn_rl_repo" not in sys.path:
    sys.path.insert(0, "/opt/trn_rl_repo")

import numpy as np

B, S, V, D = 256, 512, 100, 64
NCORES = 8
R = B // NCORES  # rows per core

# const tile column layout (f32 [128, CW])
C_W1R0 = 0
C_W1R1 = 64
C_B1 = 128
C_W2 = 256
C_ID = 320
C_VIDX = 448
C_B2C = 449
CW = 450

_CACHE = {}
LAST_RESULT = None


def _emit(ctx, nc, tc, src32, dst32, selc, consts, out):
    from concourse import bass

    mybir = bass.mybir
    f32 = mybir.dt.float32
    f32r = mybir.dt.float32r
    bf16 = mybir.dt.bfloat16
    alu = mybir.AluOpType

    consts_p = ctx.enter_context(tc.tile_pool(name="cst", bufs=1))
    oh_p = ctx.enter_context(tc.tile_pool(name="oh", bufs=6))
    trash_p = ctx.enter_context(tc.tile_pool(name="trash", bufs=2))
    hist_p = ctx.enter_context(tc.tile_pool(name="hist", bufs=6))
    mlp_p = ctx.enter_context(tc.tile_pool(name="mlp", bufs=4))
    gout_p = ctx.enter_context(tc.tile_pool(name="gout", bufs=3))
    ps_bc = ctx.enter_context(tc.tile_pool(name="ps_bc", bufs=2, space="PSUM"))
    ps_mlp = ctx.enter_context(tc.tile_pool(name="ps_mlp", bufs=1, space="PSUM"))
    ps_g = ctx.enter_context(tc.tile_pool(name="ps_g", bufs=1, space="PSUM"))

    sb_sel = consts_p.tile([R, R], bf16)
    nc.sync.dma_start(out=sb_sel, in_=selc)
    sb_src = consts_p.tile([R, S], bf16)
    nc.sync.dma_start(out=sb_src, in_=src32)
    cst = consts_p.tile([128, CW], f32)
    # 32-partition chunks: a 128-partition DMA fans out across 4 HWDGE
    # queues and consumers would need 4 sync waits (HW allows 1)
    for p in range(0, 128, 32):
        nc.scalar.dma_start(out=cst[p : p + 32, :], in_=consts[p : p + 32, :])
    sb_dst = consts_p.tile([R, S], bf16)
    nc.scalar.dma_start(out=sb_dst, in_=dst32)

    w1r0 = cst[0:V, C_W1R0 : C_W1R0 + D]
    w1r1 = cst[0:V, C_W1R1 : C_W1R1 + D]
    b1b = cst[0:V, C_B1 : C_B1 + D]
    w2 = cst[0:D, C_W2 : C_W2 + D]
    ident = cst[0:V, C_ID : C_ID + V]
    vidx = cst[:, C_VIDX : C_VIDX + 1]
    b2c = cst[:, C_B2C : C_B2C + 1]

    # startup absorbers: every engine waits each input-DMA queue sem once
    # (HW allows 1 sync wait per instruction; wide DMAs fan out over queues)
    dvedum = trash_p.tile([1, 1], f32)
    actdum = trash_p.tile([1, 1], f32)
    pooldum = trash_p.tile([1, 1], f32)
    # base partition must be in {0,32,64}: chunk 3 is reached by a 64:128
    # span once chunk 2's queue wait is already absorbed
    for p0, p1 in ((0, 1), (32, 33), (64, 65), (64, 128)):
        c1 = cst[p0:p1, 0:1]
        cd = actdum if p1 - p0 == 1 else trash_p.tile([64, 1], f32)
        nc.scalar.copy(out=cd, in_=c1)
        pd = pooldum if p1 - p0 == 1 else trash_p.tile([64, 1], f32)
        nc.gpsimd.tensor_scalar_max(pd, c1, 0.0)
        dd = dvedum if p1 - p0 == 1 else trash_p.tile([64, 1], f32)
        nc.vector.tensor_tensor(out=dd, in0=c1, in1=c1, op=alu.add)

    # FP32r matmul inputs must be produced as f32r (verifier rejects bitcasts)
    w2r = consts_p.tile([D, D], f32r)
    nc.scalar.copy(out=w2r, in_=w2)

    def bcast_pair(r):
        # replicate ids row r (src cols 0:S, dst cols S:2S) to 128 partitions:
        # lhsT[k, p] = (k == r) via identity column broadcast over free dim
        ps = ps_bc.tile([128, 2 * S], f32)
        if r == 0:
            for p0, p1 in ((0, 1), (32, 33), (64, 65), (64, 128)):
                c1 = cst[p0:p1, 0:1]
                nc.tensor.matmul(
                    out=ps[0:1, 0:1], lhsT=c1, rhs=c1, skip_group_check=True
                )
            for sb in (sb_sel, sb_src, sb_dst):
                nc.tensor.matmul(
                    out=ps[0:1, 0:1], lhsT=sb[0:1, 0:1], rhs=sb[0:1, 0:1],
                    skip_group_check=True,
                )
        lhsT = sb_sel[:, r : r + 1].broadcast_to((R, 128))
        nc.tensor.matmul(out=ps[:, 0:S], lhsT=lhsT, rhs=sb_src)
        nc.tensor.matmul(out=ps[:, S : 2 * S], lhsT=lhsT, rhs=sb_dst)
        return ps

    from collections import deque

    pend = deque()
    pend.append(bcast_pair(0))
    pend.append(bcast_pair(1))
    gout = None
    for r in range(R):
        ps_pair = pend.popleft()

        # onehot[v, s] = (ids[s] == v), per side; each half's fused
        # accumulate yields its histogram directly (Pool lacks
        # TensorScalarPtr on TRN2, so no Pool accumulate path)
        oh = oh_p.tile([128, 2 * S], f32r)
        h_s = hist_p.tile([128, 1], f32)
        h_d = hist_p.tile([128, 1], f32)
        nc.vector.memset(h_s[0:1, 0:1], 0.0)
        nc.vector.tensor_tensor(
            out=dvedum, in0=ps_pair[0:1, 0:1], in1=dvedum, op=alu.add
        )
        nc.vector.tensor_scalar(
            out=oh[:, 0:S], in0=ps_pair[:, 0:S], scalar1=vidx, scalar2=None,
            op0=alu.is_equal, op1=alu.add, accum_out=h_s,
        )
        nc.vector.tensor_scalar(
            out=oh[:, S : 2 * S], in0=ps_pair[:, S : 2 * S], scalar1=vidx,
            scalar2=None, op0=alu.is_equal, op1=alu.add, accum_out=h_d,
        )
        if r + 2 < R:
            pend.append(bcast_pair(r + 2))

        # padding id 0 contributes zero features to the MLP
        nc.gpsimd.memset(h_s[0:1, :], 0.0)
        nc.gpsimd.memset(h_d[0:1, :], 0.0)

        # table[v,:] = relu(h_s[v]*W1[0] + h_d[v]*W1[1] + b1) @ W2  (b2 folded
        # into the gout copy as a per-partition bias)
        tmp = mlp_p.tile([V, D], f32)
        nc.vector.scalar_tensor_tensor(
            out=tmp, in0=w1r0, scalar=h_s[0:V, :], in1=b1b,
            op0=alu.mult, op1=alu.add,
        )
        hpre = mlp_p.tile([V, D], f32)
        nc.vector.scalar_tensor_tensor(
            out=hpre, in0=w1r1, scalar=h_d[0:V, :], in1=tmp,
            op0=alu.mult, op1=alu.add,
        )
        hrelu = mlp_p.tile([V, D], f32)
        nc.gpsimd.tensor_scalar_max(hrelu, hpre, 0.0)
        pst = ps_mlp.tile([D, V], f32)
        nc.tensor.transpose(pst, hrelu, ident)
        hT = mlp_p.tile([D, V], f32r)
        nc.scalar.copy(out=hT, in_=pst)
        pstab = ps_mlp.tile([V, D], f32)
        nc.tensor.matmul(out=pstab, lhsT=hT, rhs=w2r)
        tab = mlp_p.tile([V, D], f32r)
        nc.scalar.copy(out=tab, in_=pstab)

        # gather each side: outT[f, s] = table[ids[s], f]; both matmul outs
        # at partition base 0 (PE rejects stationary tiles at column 64)
        ps_gs = ps_g.tile([128, S], f32)
        ps_gd = ps_g.tile([128, S], f32)
        nc.tensor.matmul(out=ps_gs[0:D, :], lhsT=tab, rhs=oh[0:V, 0:S])
        nc.tensor.matmul(out=ps_gd[0:D, :], lhsT=tab, rhs=oh[0:V, S : 2 * S])

        # psum -> sbuf with b2 added as per-partition bias; DMA every 2 rows.
        # gout free layout: [src r0 | src r1 | dst r0 | dst r1], 64 partitions
        g = r % 2
        if g == 0:
            gout = gout_p.tile([D, 4 * S], f32)
            if r >= 6:
                # absorb the SP-DMA WAR wait on the reused gout buffer
                nc.scalar.copy(out=gout[0:1, 0:1], in_=hT[0:1, 0:1])
        nc.scalar.activation(
            out=gout[:, g * S : (g + 1) * S], in_=ps_gs[0:D, :],
            func=mybir.ActivationFunctionType.Identity, bias=b2c[0:D, :],
            scale=1.0,
        )
        nc.scalar.activation(
            out=gout[:, (2 + g) * S : (3 + g) * S], in_=ps_gd[0:D, :],
            func=mybir.ActivationFunctionType.Identity, bias=b2c[0:D, :],
            scale=1.0,
        )
        if g == 1:
            win = slice((r - 1) * S, (r + 1) * S)
            nc.sync.dma_start(out=out[0:D, win], in_=gout[:, 0 : 2 * S])
            nc.sync.dma_start(
                out=out[D : 2 * D, win], in_=gout[:, 2 * S : 4 * S]
            )


def _build_module():
    from contextlib import ExitStack

    from concourse import bacc, bass, tile

    mybir = bass.mybir
    # Bacc.finalize() runs generate_event_semaphores, splitting sync waits
    # to the HW limit of 1 per instruction (raw Bass skips that pass)
    nc = bacc.Bacc(
        "TRN2", target_bir_lowering=False, debug=False, num_devices=NCORES
    )
    src32 = nc.dram_tensor(
        "src32", [R, S], mybir.dt.bfloat16, kind="ExternalInput"
    ).ap()
    dst32 = nc.dram_tensor(
        "dst32", [R, S], mybir.dt.bfloat16, kind="ExternalInput"
    ).ap()
    selc = nc.dram_tensor(
        "selc", [R, R], mybir.dt.bfloat16, kind="ExternalInput"
    ).ap()
    consts = nc.dram_tensor(
        "consts", [128, CW], mybir.dt.float32, kind="ExternalInput"
    ).ap()
    out = nc.dram_tensor(
        "out", [128, R * S], mybir.dt.float32, kind="ExternalOutput"
    ).ap()

    with tile.TileContext(nc) as tc:
        with ExitStack() as ctx:
            _emit(ctx, nc, tc, src32, dst32, selc, consts, out)
    nc.finalize()
    return nc


def get_module():
    if "nc" not in _CACHE:
        _CACHE["nc"] = _build_module()
    return _CACHE["nc"]


def _build_consts(W1, b1, W2, b2):
    c = np.zeros((128, CW), np.float32)
    c[:, C_W1R0 : C_W1R0 + D] = W1[0]
    c[:, C_W1R1 : C_W1R1 + D] = W1[1]
    c[:, C_B1 : C_B1 + D] = b1
    c[0:D, C_W2 : C_W2 + D] = W2
    c[:, C_ID : C_ID + 128] = np.eye(128, dtype=np.float32)
    c[:, C_VIDX] = np.arange(128, dtype=np.float32)
    c[0:D, C_B2C] = b2
    c[D : 2 * D, C_B2C] = b2
    return c


def _build_selc():
    import ml_dtypes

    return np.eye(R, dtype=np.float32).astype(ml_dtypes.bfloat16)


def kernel(**inputs):
    global LAST_RESULT
    import ml_dtypes

    from concourse import bass_utils

    src = np.asarray(inputs["src_neighbor_ids"])
    dst = np.asarray(inputs["dst_neighbor_ids"])
    W1 = np.asarray(inputs["W1"], np.float32)
    b1 = np.asarray(inputs["b1"], np.float32)
    W2 = np.asarray(inputs["W2"], np.float32)
    b2 = np.asarray(inputs["b2"], np.float32)

    consts = _build_consts(W1, b1, W2, b2)
    selc = _build_selc()
    bf16 = ml_dtypes.bfloat16
    src16 = src.astype(np.float32).astype(bf16)
    dst16 = dst.astype(np.float32).astype(bf16)

    in_maps = []
    for c in range(NCORES):
        sl = slice(c * R, (c + 1) * R)
        in_maps.append(
            {
                "src32": src16[sl],
                "dst32": dst16[sl],
                "selc": selc,
                "consts": consts,
            }
        )

    nc = get_module()
    import os

    trace = bool(int(os.environ.get("KERNEL_TRACE", "0")))
    res = bass_utils.run_bass_kernel_spmd(
        nc, in_maps, core_ids=list(range(NCORES)), trace=trace
    )
    LAST_RESULT = res

    src_feat = np.empty((B, S, D), np.float32)
    dst_feat = np.empty((B, S, D), np.float32)
    for c in range(NCORES):
        o = res.results[c]["out"].reshape(128, R, S)
        sl = slice(c * R, (c + 1) * R)
        src_feat[sl] = o[0:D].transpose(1, 2, 0)
        dst_feat[sl] = o[D : 2 * D].transpose(1, 2, 0)
    return src_feat, dst_feat



# revision 2
# speedup vs baseline: 1.0107x; 1.0107x over previous
import sys

if "/opt/trn_rl_repo" not in sys.path:
    sys.path.insert(0, "/opt/trn_rl_repo")

import numpy as np

B, S, V, D = 256, 512, 100, 64
NCORES = 8
R = B // NCORES  # rows per core
VP = V + 1  # gather K: vocab rows + ones/bias row

PB_BOUNDS = [0, 1024, 2048, 4096]
while PB_BOUNDS[-1] < R * 2 * S:
    PB_BOUNDS.append(min(PB_BOUNDS[-1] + 4096, R * 2 * S))
NPBCH = len(PB_BOUNDS) - 1
PBW = max(b - a for a, b in zip(PB_BOUNDS, PB_BOUNDS[1:]))

# const tile layout (bf16 [128, 356])
C_W1R0 = 0
C_W1R1 = 64
C_B1 = 128
C_W2 = 192
C_ID = 256
CW = 356

_CACHE = {}
LAST_RESULT = None


def _emit(ctx, nc, tc, pb, cst, b1c, b2row, out):
    from concourse import bass

    mybir = bass.mybir
    f32 = mybir.dt.float32
    bf16 = mybir.dt.bfloat16
    alu = mybir.AluOpType
    act = mybir.ActivationFunctionType

    consts_p = ctx.enter_context(tc.tile_pool(name="cst", bufs=1))
    oh_p = ctx.enter_context(tc.tile_pool(name="oh", bufs=1))
    hsd_p = ctx.enter_context(tc.tile_pool(name="hsd", bufs=3))
    mlp_p = ctx.enter_context(tc.tile_pool(name="mlp", bufs=3))
    hT_p = ctx.enter_context(tc.tile_pool(name="hT", bufs=2))
    tab_p = ctx.enter_context(tc.tile_pool(name="tab", bufs=1))
    gout_p = ctx.enter_context(tc.tile_pool(name="gout", bufs=2))
    ps_t = ctx.enter_context(tc.tile_pool(name="ps_t", bufs=2, space="PSUM"))
    ps_m = ctx.enter_context(tc.tile_pool(name="ps_m", bufs=2, space="PSUM"))
    ps_g = ctx.enter_context(tc.tile_pool(name="ps_g", bufs=1, space="PSUM"))

    # resident prebroadcast ids: pb_sb[v, r*1024 + side*512 + s] = ids - v
    pb_sb = consts_p.tile([V, R * 2 * S], bf16)
    for i, (a, b) in enumerate(zip(PB_BOUNDS, PB_BOUNDS[1:])):
        nc.sync.dma_start(
            out=pb_sb[:, a:b], in_=pb[i * V : (i + 1) * V, 0 : b - a]
        )
    cstt = consts_p.tile([128, CW], bf16)
    nc.scalar.dma_start(out=cstt, in_=cst)

    w1r0b = cstt[0:V, C_W1R0 : C_W1R0 + D]
    w1r1b = cstt[0:V, C_W1R1 : C_W1R1 + D]
    b1b = cstt[0:V, C_B1 : C_B1 + D]
    w2b = cstt[0:D, C_W2 : C_W2 + D]
    identb = cstt[0:V, C_ID : C_ID + V]

    oh_tiles = [oh_p.tile([VP, 2 * S], bf16, name=f"oh{i}") for i in range(4)]
    for t in oh_tiles:
        # base partition must be in {0,32,64,96}; rows 96:100 are
        # overwritten by every compare, row 100 stays 1
        nc.gpsimd.memset(t[96:VP, :], 1.0)
    tab_tiles = [tab_p.tile([VP, D], bf16, name=f"tab{i}") for i in range(2)]
    for t in tab_tiles:
        # rows 96:100 are overwritten by every tab copy, row 100 = b2
        nc.scalar.dma_start(out=t[96:VP, :], in_=b2row)

    psg_tiles = [ps_g.tile([128, 2 * S], f32, name=f"psg{i}") for i in range(2)]
    gout = None

    for r in range(R):
        oh = oh_tiles[r % 4]
        hsd = hsd_p.tile([128, 2], f32)
        base = r * 2 * S
        nc.vector.tensor_scalar(
            out=oh[0:V, 0:S], in0=pb_sb[:, base : base + S],
            scalar1=0.0, scalar2=None, op0=alu.is_equal, op1=alu.add,
            accum_out=hsd[0:V, 0:1],
        )
        nc.vector.tensor_scalar(
            out=oh[0:V, S : 2 * S], in0=pb_sb[:, base + S : base + 2 * S],
            scalar1=0.0, scalar2=None, op0=alu.is_equal, op1=alu.add,
            accum_out=hsd[0:V, 1:2],
        )
        # padding id 0 contributes encode(0, 0)
        nc.gpsimd.memset(hsd[0:1, 0:2], 0.0)

        tmp = mlp_p.tile([V, D], bf16)
        nc.vector.scalar_tensor_tensor(
            out=tmp, in0=w1r0b, scalar=hsd[0:V, 0:1], in1=b1b,
            op0=alu.mult, op1=alu.add,
        )
        hpre = mlp_p.tile([V, D], bf16)
        nc.vector.scalar_tensor_tensor(
            out=hpre, in0=w1r1b, scalar=hsd[0:V, 1:2], in1=tmp,
            op0=alu.mult, op1=alu.add,
        )
        pst = ps_t.tile([D, V], bf16)
        nc.tensor.transpose(pst, hpre, identb)
        hTr = hT_p.tile([D, V], bf16)
        nc.scalar.activation(out=hTr, in_=pst, func=act.Relu, scale=1.0)
        pstab = ps_m.tile([V, D], f32)
        nc.tensor.matmul(out=pstab, lhsT=hTr, rhs=w2b)
        tab = tab_tiles[r % 2]
        nc.scalar.activation(
            out=tab[0:V, :], in_=pstab, func=act.Identity, scale=1.0
        )

        g = r % 2
        psg = psg_tiles[(r // 2) % 2]
        win = slice(g * S, (g + 1) * S)
        nc.tensor.matmul(
            out=psg[0:D, win], lhsT=tab, rhs=oh[:, 0:S], tile_position=(0, 0)
        )
        nc.tensor.matmul(
            out=psg[D : 2 * D, win], lhsT=tab, rhs=oh[:, S : 2 * S],
            tile_position=(0, D),
        )
        if g == 1:
            q = (r // 2) % 2
            if q == 0:
                gout = gout_p.tile([128, 4 * S], bf16)
            nc.scalar.activation(
                out=gout[:, q * 2 * S : (q + 1) * 2 * S], in_=psg,
                func=act.Identity, scale=1.0,
            )
            # last batch ships per 2 rows to shorten the kernel tail
            if r == R - 3:
                nc.sync.dma_start(
                    out=out[:, (r - 1) * S : (r + 1) * S],
                    in_=gout[:, 0 : 2 * S],
                )
            elif r == R - 1:
                nc.sync.dma_start(
                    out=out[:, (r - 1) * S : (r + 1) * S],
                    in_=gout[:, 2 * S : 4 * S],
                )
            elif q == 1:
                nc.sync.dma_start(
                    out=out[:, (r - 3) * S : (r + 1) * S], in_=gout
                )


def _build_module():
    from contextlib import ExitStack

    from concourse import bacc, bass, tile

    mybir = bass.mybir
    nc = bacc.Bacc(
        "TRN2", target_bir_lowering=False, debug=False, num_devices=NCORES
    )
    pb = nc.dram_tensor(
        "pb", [NPBCH * V, PBW], mybir.dt.bfloat16, kind="ExternalInput"
    ).ap()
    cst = nc.dram_tensor(
        "cst", [128, CW], mybir.dt.bfloat16, kind="ExternalInput"
    ).ap()
    b1c = nc.dram_tensor(
        "b1c", [D, 1], mybir.dt.float32, kind="ExternalInput"
    ).ap()
    b2row = nc.dram_tensor(
        "b2row", [5, D], mybir.dt.bfloat16, kind="ExternalInput"
    ).ap()
    out = nc.dram_tensor(
        "out", [128, R * S], mybir.dt.bfloat16, kind="ExternalOutput"
    ).ap()

    with tile.TileContext(nc) as tc:
        with ExitStack() as ctx:
            _emit(ctx, nc, tc, pb, cst, b1c, b2row, out)
    nc.finalize()
    return nc


def get_module():
    if "nc" not in _CACHE:
        _CACHE["nc"] = _build_module()
    return _CACHE["nc"]


def _build_consts(W1, b1, W2):
    import ml_dtypes

    c = np.zeros((128, CW), np.float32)
    c[:, C_W1R0 : C_W1R0 + D] = W1[0]
    c[:, C_W1R1 : C_W1R1 + D] = W1[1]
    c[:, C_B1 : C_B1 + D] = b1
    c[0:D, C_W2 : C_W2 + D] = W2
    c[0:V, C_ID : C_ID + V] = np.eye(V, dtype=np.float32)
    return c.astype(ml_dtypes.bfloat16)


def _build_pb(src, dst):
    # logical pb[v, r*1024 + side*512 + s] = ids[r, s] - v (bf16 exact),
    # stored chunk-major: chunk i at rows [i*V:(i+1)*V], HBM-contiguous
    import ml_dtypes

    ids = np.stack([src, dst], axis=1).astype(np.float32)  # [R, 2, S]
    v = np.arange(V, dtype=np.float32)
    pbf = (ids.reshape(1, R * 2 * S) - v[:, None]).astype(ml_dtypes.bfloat16)
    out = np.zeros((NPBCH * V, PBW), ml_dtypes.bfloat16)
    for i, (a, b) in enumerate(zip(PB_BOUNDS, PB_BOUNDS[1:])):
        out[i * V : (i + 1) * V, 0 : b - a] = pbf[:, a:b]
    return out


def kernel(**inputs):
    global LAST_RESULT
    import ml_dtypes

    from concourse import bass_utils

    src = np.asarray(inputs["src_neighbor_ids"])
    dst = np.asarray(inputs["dst_neighbor_ids"])
    W1 = np.asarray(inputs["W1"], np.float32)
    b1 = np.asarray(inputs["b1"], np.float32)
    W2 = np.asarray(inputs["W2"], np.float32)
    b2 = np.asarray(inputs["b2"], np.float32)

    bf16 = ml_dtypes.bfloat16
    consts = _build_consts(W1, b1, W2)
    b2r = np.tile(b2.reshape(1, D), (5, 1)).astype(bf16)

    in_maps = []
    for c in range(NCORES):
        sl = slice(c * R, (c + 1) * R)
        in_maps.append(
            {
                "pb": _build_pb(src[sl], dst[sl]),
                "cst": consts,
                "b1c": b1.reshape(D, 1),
                "b2row": b2r,
            }
        )

    nc = get_module()
    import os

    trace = bool(int(os.environ.get("KERNEL_TRACE", "0")))
    res = bass_utils.run_bass_kernel_spmd(
        nc, in_maps, core_ids=list(range(NCORES)), trace=trace
    )
    LAST_RESULT = res

    src_feat = np.empty((B, S, D), np.float32)
    dst_feat = np.empty((B, S, D), np.float32)
    for c in range(NCORES):
        o = res.results[c]["out"].astype(np.float32).reshape(128, R, S)
        sl = slice(c * R, (c + 1) * R)
        src_feat[sl] = o[0:D].transpose(1, 2, 0)
        dst_feat[sl] = o[D : 2 * D].transpose(1, 2, 0)
    return src_feat, dst_feat


# revision 3
# speedup vs baseline: 1.1631x; 1.1507x over previous
import sys

if "/opt/trn_rl_repo" not in sys.path:
    sys.path.insert(0, "/opt/trn_rl_repo")

import numpy as np

B, S, V, D = 256, 512, 100, 64
NCORES = 8
R = B // NCORES  # rows per core
VP = V + 1  # gather K: vocab rows + ones/bias row

PB_BOUNDS = [0, 1024, 2048, 4096]
while PB_BOUNDS[-1] < R * 2 * S:
    PB_BOUNDS.append(min(PB_BOUNDS[-1] + 4096, R * 2 * S))
NPBCH = len(PB_BOUNDS) - 1
PBW = max(b - a for a, b in zip(PB_BOUNDS, PB_BOUNDS[1:]))

# const tile layout (bf16 [128, 356])
C_W1R0 = 0
C_W1R1 = 64
C_B1 = 128
C_W2 = 192
C_ID = 256
CW = 356

_CACHE = {}
LAST_RESULT = None


def _emit(ctx, nc, tc, pb, cst, b1c, b2row, out):
    from concourse import bass

    mybir = bass.mybir
    f32 = mybir.dt.float32
    bf16 = mybir.dt.bfloat16
    alu = mybir.AluOpType
    act = mybir.ActivationFunctionType

    consts_p = ctx.enter_context(tc.tile_pool(name="cst", bufs=1))
    oh_p = ctx.enter_context(tc.tile_pool(name="oh", bufs=1))
    hsd_p = ctx.enter_context(tc.tile_pool(name="hsd", bufs=3))
    mlp_p = ctx.enter_context(tc.tile_pool(name="mlp", bufs=3))
    hT_p = ctx.enter_context(tc.tile_pool(name="hT", bufs=2))
    tab_p = ctx.enter_context(tc.tile_pool(name="tab", bufs=1))
    gout_p = ctx.enter_context(tc.tile_pool(name="gout", bufs=2))
    ps_t = ctx.enter_context(tc.tile_pool(name="ps_t", bufs=2, space="PSUM"))
    ps_m = ctx.enter_context(tc.tile_pool(name="ps_m", bufs=2, space="PSUM"))
    ps_g = ctx.enter_context(tc.tile_pool(name="ps_g", bufs=1, space="PSUM"))

    # resident prebroadcast ids: pb_sb[v, r*1024 + side*512 + s] = ids - v
    pb_sb = consts_p.tile([V, R * 2 * S], bf16)
    for i, (a, b) in enumerate(zip(PB_BOUNDS, PB_BOUNDS[1:])):
        nc.sync.dma_start(
            out=pb_sb[:, a:b], in_=pb[i * V : (i + 1) * V, 0 : b - a]
        )
    cstt = consts_p.tile([128, CW], bf16)
    nc.scalar.dma_start(out=cstt, in_=cst)

    w1r0b = cstt[0:V, C_W1R0 : C_W1R0 + D]
    w1r1b = cstt[0:V, C_W1R1 : C_W1R1 + D]
    b1b = cstt[0:V, C_B1 : C_B1 + D]
    w2b = cstt[0:D, C_W2 : C_W2 + D]
    identb = cstt[0:V, C_ID : C_ID + V]

    oh_tiles = [oh_p.tile([VP, 2 * S], bf16, name=f"oh{i}") for i in range(4)]
    for t in oh_tiles:
        # base partition must be in {0,32,64,96}; rows 96:100 are
        # overwritten by every compare, row 100 stays 1
        nc.gpsimd.memset(t[96:VP, :], 1.0)
    tab_tiles = [tab_p.tile([VP, D], bf16, name=f"tab{i}") for i in range(2)]
    for t in tab_tiles:
        # rows 96:100 are overwritten by every tab copy, row 100 = b2
        nc.scalar.dma_start(out=t[96:VP, :], in_=b2row)

    psg_tiles = [ps_g.tile([128, 2 * S], f32, name=f"psg{i}") for i in range(2)]
    gout = None

    for r in range(R):
        oh = oh_tiles[r % 4]
        hsd = hsd_p.tile([128, 2], f32)
        base = r * 2 * S
        nc.vector.tensor_scalar(
            out=oh[0:V, 0:S], in0=pb_sb[:, base : base + S],
            scalar1=0.0, scalar2=None, op0=alu.is_equal, op1=alu.add,
            accum_out=hsd[0:V, 0:1],
        )
        nc.vector.tensor_scalar(
            out=oh[0:V, S : 2 * S], in0=pb_sb[:, base + S : base + 2 * S],
            scalar1=0.0, scalar2=None, op0=alu.is_equal, op1=alu.add,
            accum_out=hsd[0:V, 1:2],
        )
        # padding id 0 contributes encode(0, 0)
        nc.gpsimd.memset(hsd[0:1, 0:2], 0.0)

        tmp = mlp_p.tile([V, D], bf16)
        nc.vector.scalar_tensor_tensor(
            out=tmp, in0=w1r0b, scalar=hsd[0:V, 0:1], in1=b1b,
            op0=alu.mult, op1=alu.add,
        )
        hpre = mlp_p.tile([V, D], bf16)
        nc.vector.scalar_tensor_tensor(
            out=hpre, in0=w1r1b, scalar=hsd[0:V, 1:2], in1=tmp,
            op0=alu.mult, op1=alu.add,
        )
        pst = ps_t.tile([D, V], bf16)
        nc.tensor.transpose(pst, hpre, identb)
        hTr = hT_p.tile([D, V], bf16)
        nc.scalar.activation(out=hTr, in_=pst, func=act.Relu, scale=1.0)
        pstab = ps_m.tile([V, D], f32)
        nc.tensor.matmul(out=pstab, lhsT=hTr, rhs=w2b)
        tab = tab_tiles[r % 2]
        nc.scalar.activation(
            out=tab[0:V, :], in_=pstab, func=act.Identity, scale=1.0
        )

        g = r % 2
        psg = psg_tiles[(r // 2) % 2]
        win = slice(g * S, (g + 1) * S)
        nc.tensor.matmul(
            out=psg[0:D, win], lhsT=tab, rhs=oh[:, 0:S], tile_position=(0, 0)
        )
        nc.tensor.matmul(
            out=psg[D : 2 * D, win], lhsT=tab, rhs=oh[:, S : 2 * S],
            tile_position=(0, D),
        )
        if g == 1:
            q = (r // 2) % 2
            if q == 0:
                gout = gout_p.tile([128, 4 * S], bf16)
            nc.scalar.activation(
                out=gout[:, q * 2 * S : (q + 1) * 2 * S], in_=psg,
                func=act.Identity, scale=1.0,
            )
            # last batch ships per 2 rows to shorten the kernel tail
            if r == R - 3:
                nc.gpsimd.dma_start(
                    out=out[:, (r - 1) * S : (r + 1) * S],
                    in_=gout[:, 0 : 2 * S],
                )
            elif r == R - 1:
                nc.gpsimd.dma_start(
                    out=out[:, (r - 1) * S : (r + 1) * S],
                    in_=gout[:, 2 * S : 4 * S],
                )
            elif q == 1:
                nc.gpsimd.dma_start(
                    out=out[:, (r - 3) * S : (r + 1) * S], in_=gout
                )


def _build_module():
    from contextlib import ExitStack

    from concourse import bacc, bass, tile

    mybir = bass.mybir
    nc = bacc.Bacc(
        "TRN2", target_bir_lowering=False, debug=False, num_devices=NCORES
    )
    pb = nc.dram_tensor(
        "pb", [NPBCH * V, PBW], mybir.dt.bfloat16, kind="ExternalInput"
    ).ap()
    cst = nc.dram_tensor(
        "cst", [128, CW], mybir.dt.bfloat16, kind="ExternalInput"
    ).ap()
    b1c = nc.dram_tensor(
        "b1c", [D, 1], mybir.dt.float32, kind="ExternalInput"
    ).ap()
    b2row = nc.dram_tensor(
        "b2row", [5, D], mybir.dt.bfloat16, kind="ExternalInput"
    ).ap()
    out = nc.dram_tensor(
        "out", [128, R * S], mybir.dt.bfloat16, kind="ExternalOutput"
    ).ap()

    with tile.TileContext(nc) as tc:
        with ExitStack() as ctx:
            _emit(ctx, nc, tc, pb, cst, b1c, b2row, out)
    nc.finalize()
    return nc


def get_module():
    if "nc" not in _CACHE:
        _CACHE["nc"] = _build_module()
    return _CACHE["nc"]


def _build_consts(W1, b1, W2):
    import ml_dtypes

    c = np.zeros((128, CW), np.float32)
    c[:, C_W1R0 : C_W1R0 + D] = W1[0]
    c[:, C_W1R1 : C_W1R1 + D] = W1[1]
    c[:, C_B1 : C_B1 + D] = b1
    c[0:D, C_W2 : C_W2 + D] = W2
    c[0:V, C_ID : C_ID + V] = np.eye(V, dtype=np.float32)
    return c.astype(ml_dtypes.bfloat16)


def _build_pb(src, dst):
    # logical pb[v, r*1024 + side*512 + s] = ids[r, s] - v (bf16 exact),
    # stored chunk-major: chunk i at rows [i*V:(i+1)*V], HBM-contiguous
    import ml_dtypes

    ids = np.stack([src, dst], axis=1).astype(np.float32)  # [R, 2, S]
    v = np.arange(V, dtype=np.float32)
    pbf = (ids.reshape(1, R * 2 * S) - v[:, None]).astype(ml_dtypes.bfloat16)
    out = np.zeros((NPBCH * V, PBW), ml_dtypes.bfloat16)
    for i, (a, b) in enumerate(zip(PB_BOUNDS, PB_BOUNDS[1:])):
        out[i * V : (i + 1) * V, 0 : b - a] = pbf[:, a:b]
    return out


def kernel(**inputs):
    global LAST_RESULT
    import ml_dtypes

    from concourse import bass_utils

    src = np.asarray(inputs["src_neighbor_ids"])
    dst = np.asarray(inputs["dst_neighbor_ids"])
    W1 = np.asarray(inputs["W1"], np.float32)
    b1 = np.asarray(inputs["b1"], np.float32)
    W2 = np.asarray(inputs["W2"], np.float32)
    b2 = np.asarray(inputs["b2"], np.float32)

    bf16 = ml_dtypes.bfloat16
    consts = _build_consts(W1, b1, W2)
    b2r = np.tile(b2.reshape(1, D), (5, 1)).astype(bf16)

    in_maps = []
    for c in range(NCORES):
        sl = slice(c * R, (c + 1) * R)
        in_maps.append(
            {
                "pb": _build_pb(src[sl], dst[sl]),
                "cst": consts,
                "b1c": b1.reshape(D, 1),
                "b2row": b2r,
            }
        )

    nc = get_module()
    import os

    trace = bool(int(os.environ.get("KERNEL_TRACE", "0")))
    res = bass_utils.run_bass_kernel_spmd(
        nc, in_maps, core_ids=list(range(NCORES)), trace=trace
    )
    LAST_RESULT = res

    src_feat = np.empty((B, S, D), np.float32)
    dst_feat = np.empty((B, S, D), np.float32)
    for c in range(NCORES):
        o = res.results[c]["out"].astype(np.float32).reshape(128, R, S)
        sl = slice(c * R, (c + 1) * R)
        src_feat[sl] = o[0:D].transpose(1, 2, 0)
        dst_feat[sl] = o[D : 2 * D].transpose(1, 2, 0)
    return src_feat, dst_feat
